# revision 2
# baseline (speedup 1.0000x reference)
"""LoRO sparse linear (2:4 soft-threshold low-rank) Trainium2 kernel.

out = ((x @ sw_in.T) @ sw_out.T + bias) / rank, computed in fp16 with fp32
accumulate, where sw_* = soft_threshold24(weight_*) * scale_*.

Sharding: data-parallel over the 8192 batch*seq rows across 8 cores
(1024 rows each); the rank-64 weights are replicated. Each core:
  - preprocess weights on-chip: sw = max(s*w, s*t) + min(s*w, -s*t) per
    2:4 group (t = 2nd-smallest |w| of each group of 4), PE-transpose to
    put the contraction dims on partitions.
  - stream x row-tiles [128, 4096]: PE-transpose to xT (fp16), mm1
    accumulates xpT[64, 128] over 32 K-chunks, mm2 [65, 128] x [65, 512]
    (row 64 carries ones/bias so bias fuses into the matmul), scale by
    1/rank on the PSUM->SBUF copy, store.
"""

import numpy as np

import concourse.bass as bass
import concourse.tile as tile
from concourse import bacc, mybir
from concourse.bass_utils import run_bass_kernel_spmd
from concourse.masks import make_identity

N_CORES = 8
ROWS, IN_F, OUT_F, RANK = 1024, 4096, 4096, 64  # per-core rows
F32, F16 = mybir.dt.float32, mybir.dt.float16

_CACHE: dict = {}


def _soft_threshold_scaled(nc, pool, w, P, G, s, tag):
    """w: [P, 4*G] f32 tile of 2:4 groups along free dim. Returns sw tile
    [P, 4*G] f32 with sw = s * (sign(w)*relu(|w| - t)), t = 2nd-smallest
    |w| per group. Identity used: sign(w)relu(|w|-t) = max(w,t)+min(w,-t)."""
    AT = mybir.ActivationFunctionType
    OP = mybir.AluOpType
    m = pool.tile([P, 4 * G], F32, tag=f"m_{tag}")
    nc.scalar.activation(m[:], w[:], AT.Abs)
    w4 = w[:].rearrange("p (g f) -> p f g", f=4)
    m4 = m[:].rearrange("p (g f) -> p f g", f=4)
    lo1 = pool.tile([P, G], F32, tag=f"lo1_{tag}")
    hi1 = pool.tile([P, G], F32, tag=f"hi1_{tag}")
    lo2 = pool.tile([P, G], F32, tag=f"lo2_{tag}")
    hi2 = pool.tile([P, G], F32, tag=f"hi2_{tag}")
    nc.vector.tensor_tensor(lo1[:], m4[:, 0, :], m4[:, 1, :], op=OP.min)
    nc.vector.tensor_tensor(hi1[:], m4[:, 0, :], m4[:, 1, :], op=OP.max)
    nc.vector.tensor_tensor(lo2[:], m4[:, 2, :], m4[:, 3, :], op=OP.min)
    nc.vector.tensor_tensor(hi2[:], m4[:, 2, :], m4[:, 3, :], op=OP.max)
    # t = min(max(lo1, lo2), min(hi1, hi2)) = 2nd smallest of the four
    nc.vector.tensor_tensor(lo1[:], lo1[:], lo2[:], op=OP.max)
    nc.vector.tensor_tensor(hi1[:], hi1[:], hi2[:], op=OP.min)
    t = pool.tile([P, G], F32, tag=f"t_{tag}")
    nc.vector.tensor_tensor(t[:], lo1[:], hi1[:], op=OP.min)
    ts = pool.tile([P, G], F32, tag=f"ts_{tag}")
    nts = pool.tile([P, G], F32, tag=f"nts_{tag}")
    nc.vector.tensor_scalar_mul(ts[:], t[:], float(s))
    nc.vector.tensor_scalar_mul(nts[:], t[:], float(-s))
    sw = pool.tile([P, 4 * G], F32, tag=f"sw_{tag}")
    sw4 = sw[:].rearrange("p (g f) -> p f g", f=4)
    a = pool.tile([P, G], F32, tag=f"a_{tag}")
    b = pool.tile([P, G], F32, tag=f"b_{tag}")
    # s*max(w,t) = max(s*w, s*t) for s>=0, else min(s*w, s*t); likewise
    # s*min(w,-t) flips to max for s<0.
    op_a, op_b = (OP.max, OP.min) if s >= 0 else (OP.min, OP.max)
    for j in range(4):
        nc.vector.scalar_tensor_tensor(a[:], w4[:, j, :], float(s), ts[:], OP.mult, op_a)
        nc.vector.scalar_tensor_tensor(b[:], w4[:, j, :], float(s), nts[:], OP.mult, op_b)
        nc.vector.tensor_tensor(sw4[:, j, :], a[:], b[:], op=OP.add)
    return sw


def _build(scale_in: float, scale_out: float):
    AT = mybir.ActivationFunctionType
    nc = bacc.Bacc("TRN2", target_bir_lowering=False, debug=False, enable_asserts=False)
    x_d = nc.dram_tensor("x", (ROWS, IN_F), F32, kind="ExternalInput")
    win_d = nc.dram_tensor("weight_in", (RANK, IN_F), F32, kind="ExternalInput")
    wout_d = nc.dram_tensor("weight_out", (OUT_F, RANK), F32, kind="ExternalInput")
    bias_d = nc.dram_tensor("bias", (1, OUT_F), F32, kind="ExternalInput")
    out_d = nc.dram_tensor("out", (ROWS, OUT_F), F32, kind="ExternalOutput")

    with tile.TileContext(nc) as tc:
        with (
            tc.tile_pool(name="const", bufs=1) as cpool,
            tc.tile_pool(name="wpers", bufs=1) as wpool,
        ):
            ident = cpool.tile([128, 128], F32)
            make_identity(nc, ident[:])
            # persistent weight operands for the two matmuls
            sw_inT = wpool.tile([128, 32 * RANK], F16)  # chunk k: [:, k*64:(k+1)*64]
            sw_outT = wpool.tile([RANK + 1, OUT_F], F16)  # row 64 = bias
            nc.gpsimd.dma_start(sw_outT[RANK : RANK + 1, :], bias_d.ap())

            with (
                tc.tile_pool(name="prep", bufs=1) as ppool,
                tc.tile_pool(name="prep_ps", bufs=2, space="PSUM") as ppsum,
            ):
                # --- weight_in: natural [64, 4096], groups along in_f ---
                w_in = ppool.tile([RANK, IN_F], F32)
                nc.sync.dma_start(w_in[:], win_d.ap())
                sw_in = _soft_threshold_scaled(nc, ppool, w_in, RANK, IN_F // 4, scale_in, "wi")
                # transpose to [128 in_f, 64 rank] chunks, 4 per psum tile
                for g in range(8):
                    ps = ppsum.tile([128, 4 * RANK], F32, tag="ps_wi")
                    for c in range(4):
                        k = g * 4 + c
                        nc.tensor.transpose(
                            ps[:, c * RANK : (c + 1) * RANK],
                            sw_in[:, k * 128 : (k + 1) * 128],
                            ident[:RANK, :RANK],
                        )
                    nc.vector.tensor_copy(
                        sw_inT[:, g * 4 * RANK : (g + 1) * 4 * RANK], ps[:]
                    )

                # --- weight_out: folded [128, 32*64], groups along rank ---
                w_out = ppool.tile([128, 32 * RANK], F32)
                nc.sync.dma_start(
                    w_out[:].rearrange("p (t c) -> p t c", c=RANK),
                    wout_d.ap().rearrange("(t p) c -> p t c", p=128),
                )
                sw_o = _soft_threshold_scaled(nc, ppool, w_out, 128, 32 * RANK // 4, scale_out, "wo")
                for g in range(8):
                    ps = ppsum.tile([RANK, 4 * 128], F32, tag="ps_wo")
                    for c in range(4):
                        t_ = g * 4 + c
                        nc.tensor.transpose(
                            ps[:, c * 128 : (c + 1) * 128],
                            sw_o[:, t_ * RANK : (t_ + 1) * RANK],
                            ident[:],
                        )
                    nc.vector.tensor_copy(
                        sw_outT[:RANK, g * 512 : (g + 1) * 512], ps[:]
                    )

            with (
                tc.tile_pool(name="xin", bufs=3) as xpool,
                tc.tile_pool(name="xt", bufs=2) as xtpool,
                tc.tile_pool(name="xp", bufs=2) as xppool,
                tc.tile_pool(name="outp", bufs=2) as opool,
                tc.tile_pool(name="ps_tp", bufs=2, space="PSUM") as tp_psum,
                tc.tile_pool(name="ps_mm1", bufs=2, space="PSUM") as mm1_psum,
                tc.tile_pool(name="ps_mm2", bufs=3, space="PSUM") as mm2_psum,
            ):
                for r in range(ROWS // 128):
                    x_sb = xpool.tile([128, IN_F], F32, tag="x")
                    nc.sync.dma_start(x_sb[:], x_d.ap()[r * 128 : (r + 1) * 128, :])

                    xT = xtpool.tile([128, IN_F], F16, tag="xT")
                    for b in range(8):
                        ps = tp_psum.tile([128, 512], F32, tag="tp")
                        for c in range(4):
                            k = b * 4 + c
                            nc.tensor.transpose(
                                ps[:, c * 128 : (c + 1) * 128],
                                x_sb[:, k * 128 : (k + 1) * 128],
                                ident[:],
                            )
                        nc.vector.tensor_copy(xT[:, b * 512 : (b + 1) * 512], ps[:])

                    ps_xp = mm1_psum.tile([RANK, 128], F32, tag="mm1")
                    for k in range(32):
                        nc.tensor.matmul(
                            ps_xp[:],
                            sw_inT[:, k * RANK : (k + 1) * RANK],
                            xT[:, k * 128 : (k + 1) * 128],
                            start=(k == 0),
                            stop=(k == 31),
                        )
                    xpT = xppool.tile([RANK + 1, 128], F16, tag="xpT")
                    nc.vector.tensor_copy(xpT[:RANK, :], ps_xp[:])
                    nc.vector.memset(xpT[RANK : RANK + 1, :], 1.0)

                    o_sb = opool.tile([128, OUT_F], F32, tag="o")
                    for f in range(8):
                        ps_o = mm2_psum.tile([128, 512], F32, tag="mm2")
                        nc.tensor.matmul(
                            ps_o[:],
                            xpT[:],
                            sw_outT[:, f * 512 : (f + 1) * 512],
                            start=True,
                            stop=True,
                        )
                        nc.scalar.activation(
                            o_sb[:, f * 512 : (f + 1) * 512],
                            ps_o[:],
                            AT.Copy,
                            scale=1.0 / RANK,
                        )
                    nc.sync.dma_start(out_d.ap()[r * 128 : (r + 1) * 128, :], o_sb[:])

    nc.compile()
    return nc


def kernel(x, weight_in, weight_out, bias, scale_in, scale_out):
    x = np.ascontiguousarray(np.asarray(x, dtype=np.float32)).reshape(-1, IN_F)
    weight_in = np.ascontiguousarray(np.asarray(weight_in, dtype=np.float32))
    weight_out = np.ascontiguousarray(np.asarray(weight_out, dtype=np.float32))
    bias2d = np.ascontiguousarray(np.asarray(bias, dtype=np.float32)).reshape(1, OUT_F)
    s_in, s_out = float(np.asarray(scale_in)), float(np.asarray(scale_out))

    key = (s_in, s_out)
    if key not in _CACHE:
        _CACHE[key] = _build(s_in, s_out)
    nc = _CACHE[key]

    n_rows = x.shape[0]
    assert n_rows == N_CORES * ROWS
    in_maps = [
        {
            "x": x[i * ROWS : (i + 1) * ROWS],
            "weight_in": weight_in,
            "weight_out": weight_out,
            "bias": bias2d,
        }
        for i in range(N_CORES)
    ]
    res = run_bass_kernel_spmd(nc, in_maps, core_ids=list(range(N_CORES)))
    out = np.concatenate([res.results[i]["out"] for i in range(N_CORES)], axis=0)
    return out.reshape(4, 2048, OUT_F)


# revision 20
# speedup vs baseline: 1.1420x; 1.1420x over previous
"""LoRO sparse linear (2:4 soft-threshold low-rank) Trainium2 kernel.

out = ((x @ sw_in.T) @ sw_out.T + bias) / rank, computed in fp16 with fp32
accumulate, where sw_* = soft_threshold24(weight_*) * scale_*.

Sharding: data-parallel over the 8192 batch*seq rows across 8 cores
(1024 rows each); the rank-64 weights are replicated. Each core:
  - preprocess weights on-chip: sw = max(s*w, s*t) + min(s*w, -s*t) per
    2:4 group (t = 2nd-smallest |w| of each group of 4), PE-transpose to
    put the contraction dims on partitions.
  - stream x row-tiles [128, 4096]: PE-transpose to xT (fp16), mm1
    accumulates xpT[64, 128] over 32 K-chunks, mm2 [65, 128] x [65, 512]
    (row 64 carries ones/bias so bias fuses into the matmul), scale by
    1/rank on the PSUM->SBUF copy, store.
"""

import numpy as np

import concourse.bass as bass
import concourse.tile as tile
from concourse import bacc, mybir
from concourse.bass_utils import run_bass_kernel_spmd
from concourse.masks import make_identity

N_CORES = 8
ROWS, IN_F, OUT_F, RANK = 1024, 4096, 4096, 64  # per-core rows
F32, F16 = mybir.dt.float32, mybir.dt.float16

_CACHE: dict = {}


def _soft_threshold_scaled(nc, pool, w, P, G, s, tag):
    """w: [P, 4*G] f32 tile of 2:4 groups along free dim. Returns sw tile
    [P, 4*G] f32 with sw = s * (sign(w)*relu(|w| - t)), t = 2nd-smallest
    |w| per group. Identity used: sign(w)relu(|w|-t) = max(w,t)+min(w,-t)."""
    AT = mybir.ActivationFunctionType
    OP = mybir.AluOpType
    m = pool.tile([P, 4 * G], F32, tag=f"m_{tag}")
    nc.scalar.activation(m[:], w[:], AT.Abs)
    w4 = w[:].rearrange("p (g f) -> p f g", f=4)
    m4 = m[:].rearrange("p (g f) -> p f g", f=4)
    lo1 = pool.tile([P, G], F32, tag=f"lo1_{tag}")
    hi1 = pool.tile([P, G], F32, tag=f"hi1_{tag}")
    lo2 = pool.tile([P, G], F32, tag=f"lo2_{tag}")
    hi2 = pool.tile([P, G], F32, tag=f"hi2_{tag}")
    nc.vector.tensor_tensor(lo1[:], m4[:, 0, :], m4[:, 1, :], op=OP.min)
    nc.vector.tensor_tensor(hi1[:], m4[:, 0, :], m4[:, 1, :], op=OP.max)
    nc.vector.tensor_tensor(lo2[:], m4[:, 2, :], m4[:, 3, :], op=OP.min)
    nc.vector.tensor_tensor(hi2[:], m4[:, 2, :], m4[:, 3, :], op=OP.max)
    # t = min(max(lo1, lo2), min(hi1, hi2)) = 2nd smallest of the four
    nc.vector.tensor_tensor(lo1[:], lo1[:], lo2[:], op=OP.max)
    nc.vector.tensor_tensor(hi1[:], hi1[:], hi2[:], op=OP.min)
    t = pool.tile([P, G], F32, tag=f"t_{tag}")
    nc.vector.tensor_tensor(t[:], lo1[:], hi1[:], op=OP.min)
    ts = pool.tile([P, G], F32, tag=f"ts_{tag}")
    nts = pool.tile([P, G], F32, tag=f"nts_{tag}")
    nc.vector.tensor_scalar_mul(ts[:], t[:], float(s))
    nc.vector.tensor_scalar_mul(nts[:], t[:], float(-s))
    sw = pool.tile([P, 4 * G], F32, tag=f"sw_{tag}")
    sw4 = sw[:].rearrange("p (g f) -> p f g", f=4)
    a = pool.tile([P, G], F32, tag=f"a_{tag}")
    b = pool.tile([P, G], F32, tag=f"b_{tag}")
    # s*max(w,t) = max(s*w, s*t) for s>=0, else min(s*w, s*t); likewise
    # s*min(w,-t) flips to max for s<0.
    op_a, op_b = (OP.max, OP.min) if s >= 0 else (OP.min, OP.max)
    for j in range(4):
        nc.vector.scalar_tensor_tensor(a[:], w4[:, j, :], float(s), ts[:], OP.mult, op_a)
        nc.vector.scalar_tensor_tensor(b[:], w4[:, j, :], float(s), nts[:], OP.mult, op_b)
        nc.vector.tensor_tensor(sw4[:, j, :], a[:], b[:], op=OP.add)
    return sw


def _build(scale_in: float, scale_out: float):
    AT = mybir.ActivationFunctionType
    nc = bacc.Bacc("TRN2", target_bir_lowering=False, debug=False, enable_asserts=False)
    x_d = nc.dram_tensor("x", (ROWS, IN_F), F32, kind="ExternalInput")
    win_d = nc.dram_tensor("weight_in", (RANK, IN_F), F32, kind="ExternalInput")
    wout_d = nc.dram_tensor("weight_out", (OUT_F, RANK), F32, kind="ExternalInput")
    bias_d = nc.dram_tensor("bias", (1, OUT_F), F32, kind="ExternalInput")
    out_d = nc.dram_tensor("out", (ROWS, OUT_F), F32, kind="ExternalOutput")

    with tile.TileContext(nc) as tc:
        with (
            tc.tile_pool(name="const", bufs=1) as cpool,
            tc.tile_pool(name="wpers", bufs=1) as wpool,
        ):
            ident = cpool.tile([128, 128], F32)
            make_identity(nc, ident[:])
            # persistent weight operands for the two matmuls
            sw_inT = wpool.tile([128, 32 * RANK], F16)  # chunk k: [:, k*64:(k+1)*64]
            sw_outT = wpool.tile([RANK + 1, OUT_F], F16)  # row 64 = bias
            nc.gpsimd.dma_start(sw_outT[RANK : RANK + 1, :], bias_d.ap())

            with (
                tc.tile_pool(name="prep", bufs=1) as ppool,
                tc.tile_pool(name="prep_ps", bufs=2, space="PSUM") as ppsum,
            ):
                # --- weight_in: natural [64, 4096], groups along in_f ---
                w_in = ppool.tile([RANK, IN_F], F32)
                nc.sync.dma_start(w_in[:], win_d.ap())
                sw_in = _soft_threshold_scaled(nc, ppool, w_in, RANK, IN_F // 4, scale_in, "wi")
                # transpose to [128 in_f, 64 rank] chunks, 4 per psum tile
                for g in range(8):
                    ps = ppsum.tile([128, 4 * RANK], F32, tag="ps_wi")
                    for c in range(4):
                        k = g * 4 + c
                        nc.tensor.transpose(
                            ps[:, c * RANK : (c + 1) * RANK],
                            sw_in[:, k * 128 : (k + 1) * 128],
                            ident[:RANK, :RANK],
                        )
                    nc.vector.tensor_copy(
                        sw_inT[:, g * 4 * RANK : (g + 1) * 4 * RANK], ps[:]
                    )

                # --- weight_out: folded [128, 32*64], groups along rank ---
                w_out = ppool.tile([128, 32 * RANK], F32)
                nc.sync.dma_start(
                    w_out[:].rearrange("p (t c) -> p t c", c=RANK),
                    wout_d.ap().rearrange("(t p) c -> p t c", p=128),
                )
                sw_o = _soft_threshold_scaled(nc, ppool, w_out, 128, 32 * RANK // 4, scale_out, "wo")
                for g in range(8):
                    ps = ppsum.tile([RANK, 4 * 128], F32, tag="ps_wo")
                    for c in range(4):
                        t_ = g * 4 + c
                        nc.tensor.transpose(
                            ps[:, c * 128 : (c + 1) * 128],
                            sw_o[:, t_ * RANK : (t_ + 1) * RANK],
                            ident[:],
                        )
                    nc.vector.tensor_copy(
                        sw_outT[:RANK, g * 512 : (g + 1) * 512], ps[:]
                    )

            with (
                tc.tile_pool(name="xin", bufs=3) as xpool,
                tc.tile_pool(name="xt", bufs=2) as xtpool,
                tc.tile_pool(name="xp", bufs=2) as xppool,
                tc.tile_pool(name="outp", bufs=2) as opool,
                tc.tile_pool(name="ps_tp", bufs=2, space="PSUM") as tp_psum,
                tc.tile_pool(name="ps_mm1", bufs=2, space="PSUM") as mm1_psum,
                tc.tile_pool(name="ps_mm2", bufs=3, space="PSUM") as mm2_psum,
            ):
                for r in range(ROWS // 128):
                    x_sb = xpool.tile([128, IN_F], F32, tag="x")
                    nc.sync.dma_start(x_sb[:], x_d.ap()[r * 128 : (r + 1) * 128, :])

                    xT = xtpool.tile([128, IN_F], F16, tag="xT")
                    for b in range(8):
                        ps = tp_psum.tile([128, 512], F32, tag="tp")
                        for c in range(4):
                            k = b * 4 + c
                            nc.tensor.transpose(
                                ps[:, c * 128 : (c + 1) * 128],
                                x_sb[:, k * 128 : (k + 1) * 128],
                                ident[:],
                            )
                        nc.vector.tensor_copy(xT[:, b * 512 : (b + 1) * 512], ps[:])

                    ps_xp = mm1_psum.tile([RANK, 128], F32, tag="mm1")
                    for k in range(32):
                        nc.tensor.matmul(
                            ps_xp[:],
                            sw_inT[:, k * RANK : (k + 1) * RANK],
                            xT[:, k * 128 : (k + 1) * 128],
                            start=(k == 0),
                            stop=(k == 31),
                        )
                    xpT = xppool.tile([RANK + 1, 128], F16, tag="xpT")
                    nc.vector.tensor_copy(xpT[:RANK, :], ps_xp[:])
                    nc.vector.memset(xpT[RANK : RANK + 1, :], 1.0)

                    o_sb = opool.tile([128, OUT_F], F32, tag="o")
                    for f in range(8):
                        ps_o = mm2_psum.tile([128, 512], F32, tag="mm2")
                        nc.tensor.matmul(
                            ps_o[:],
                            xpT[:],
                            sw_outT[:, f * 512 : (f + 1) * 512],
                            start=True,
                            stop=True,
                        )
                        nc.scalar.activation(
                            o_sb[:, f * 512 : (f + 1) * 512],
                            ps_o[:],
                            AT.Copy,
                            scale=1.0 / RANK,
                        )
                    nc.sync.dma_start(out_d.ap()[r * 128 : (r + 1) * 128, :], o_sb[:])

    nc.compile()
    return nc


def kernel(x, weight_in, weight_out, bias, scale_in, scale_out):
    x = np.ascontiguousarray(np.asarray(x, dtype=np.float32)).reshape(-1, IN_F)
    weight_in = np.ascontiguousarray(np.asarray(weight_in, dtype=np.float32))
    weight_out = np.ascontiguousarray(np.asarray(weight_out, dtype=np.float32))
    bias2d = np.ascontiguousarray(np.asarray(bias, dtype=np.float32)).reshape(1, OUT_F)
    s_in, s_out = float(np.asarray(scale_in)), float(np.asarray(scale_out))

    key = (s_in, s_out)
    if key not in _CACHE:
        _CACHE[key] = _build(s_in, s_out)
    nc = _CACHE[key]

    n_rows = x.shape[0]
    assert n_rows == N_CORES * ROWS
    in_maps = [
        {
            "x": x[i * ROWS : (i + 1) * ROWS],
            "weight_in": weight_in,
            "weight_out": weight_out,
            "bias": bias2d,
        }
        for i in range(N_CORES)
    ]
    res = run_bass_kernel_spmd(nc, in_maps, core_ids=list(range(N_CORES)))
    out = np.concatenate([res.results[i]["out"] for i in range(N_CORES)], axis=0)
    return out.reshape(4, 2048, OUT_F)
